# revision 8
# baseline (speedup 1.0000x reference)
"""AGNN layer (gnn_message_passing) distributed Bass kernel for 8 TRN2 NeuronCores.

Strategy (self-contained; shapes hardcoded):
- Shard edges by src-node block: core c owns src in [c*6250, (c+1)*6250).
  The scatter-add (segment_sum over src) then stays core-local: agg is
  accumulated on-chip in SBUF via dma_scatter_add's parity-split CCE mode.
  No collectives at all.
- Within a core, edges are grouped by dst bank (dst < 25000 vs >= 25000) so
  dma_gather's int16 indices reach every row from a static per-bank base.
  Each (core, bank) edge list is padded with dummy edges (valid junk indices,
  zero payload) to a fixed size so all 8 cores run one identical SPMD graph.
- h rows are gathered in bf16 with transpose-mode dma_gather, which lands
  them as [D(partitions) x edges(free)] — directly usable as the matmul
  stationary operand. Qh[src], Rh[dst], Vh[dst] are then computed on the fly
  (dense per-edge matmuls) instead of materializing N x D tables in HBM.
- e is staged host-side as transposed bf16 (e_T) so the P-matmul needs no
  on-chip transpose; e_new is produced transposed (bf16) and un-transposed
  on the host. LayerNorm runs in [edges, D] orientation (free-axis stats);
  the MLP runs in [D, edges] orientation via one PE transpose per tile.
- ln_e affine params are folded into emlp_w1/b1 on the host (exact algebra).
"""

import numpy as np
import ml_dtypes

N, E, D = 50000, 400000, 128
NCORES = 8
NSLICE = N // NCORES            # 6250
NSLICE_PAD = 6272               # 49 * 128
BANK = 25000                    # dst bank boundary (int16-reachable)
JUNK = NSLICE_PAD - 1           # junk scatter row for padded edges
TILE = 512                      # compute tile (edges)
CHUNK = 2560                    # gather chunk (edges)
WSCAT = 1280                    # scatter window (unique src per window)
NGROUPS = 25                    # agg parity groups: covers 25*256=6400 >= 6272

BF16 = ml_dtypes.bfloat16

_cache: dict = {}

# debug feature flags (bisection)
SKIP_GATHERS = False
SKIP_SCATTER = False
SKIP_PHASE2 = False
SKIP_MLP = False


def _wrap16(ix):
    """idx j -> [j%16, j//16] int16, replicated to 128 partitions."""
    n = ix.shape[-1]
    a = ix.reshape(-1, n // 16, 16).swapaxes(-1, -2)      # [..., 16, n//16]
    return np.ascontiguousarray(
        np.tile(a, (1, 8, 1)).astype(np.int16))            # [..., 128, n//16]


def _build(eb_pad, lnh_trivial):
    from concourse import bacc, tile, mybir

    epad = 2 * eb_pad
    nch = epad // CHUNK
    chunks_per_bank = eb_pad // CHUNK
    f32, bf16, i16 = mybir.dt.float32, mybir.dt.bfloat16, mybir.dt.int16
    AF = mybir.ActivationFunctionType
    ALU = mybir.AluOpType

    nc = bacc.Bacc(None, target_bir_lowering=False, debug=False)

    # ---- DRAM parameters ----
    eT_d = nc.dram_tensor("eT", [128, epad], bf16, kind="ExternalInput")
    hbf_d = nc.dram_tensor("hbf", [N, D], bf16, kind="ExternalInput")
    hsb_d = nc.dram_tensor("hsb", [NSLICE_PAD, D], bf16, kind="ExternalInput")
    sidx_d = nc.dram_tensor("sidx", [128, epad // 16], i16, kind="ExternalInput")
    didx_d = nc.dram_tensor("didx", [128, epad // 16], i16, kind="ExternalInput")
    hsT_d = nc.dram_tensor("hsT", [128, NSLICE_PAD], f32, kind="ExternalInput")
    hs_d = nc.dram_tensor("hs", [NSLICE_PAD, D], f32, kind="ExternalInput")
    wb_names = ["PT", "QT", "RT", "VT", "w1T", "w2T", "Ibf"]
    wb_d = {k: nc.dram_tensor(k, [D, D], bf16, kind="ExternalInput") for k in wb_names}
    wf_names = ["UT", "tw1T", "tw2T"]
    wf_d = {k: nc.dram_tensor(k, [D, D], f32, kind="ExternalInput") for k in wf_names}
    col_names = ["b1c", "tembc", "tb1c", "cb2c", "epsc"]
    col_d = {k: nc.dram_tensor(k, [D, 1], f32, kind="ExternalInput") for k in col_names}
    if not lnh_trivial:
        ghb_d = nc.dram_tensor("ghb", [D, D], f32, kind="ExternalInput")
        bhb_d = nc.dram_tensor("bhb", [D, D], f32, kind="ExternalInput")

    enewT_d = nc.dram_tensor("enewT", [128, epad], bf16, kind="ExternalOutput")
    hnew_d = nc.dram_tensor("hnew", [NSLICE_PAD, D], f32, kind="ExternalOutput")

    with tile.TileContext(nc) as tc:
        with tc.tile_pool(name="const", bufs=1) as cp:
            wb = {}
            for k in wb_names:
                wb[k] = cp.tile([D, D], bf16, tag=k, name=k)
                nc.sync.dma_start(out=wb[k][:], in_=wb_d[k][:])
            wf = {}
            for k in wf_names:
                wf[k] = cp.tile([D, D], f32, tag=k, name=k)
                nc.sync.dma_start(out=wf[k][:], in_=wf_d[k][:])
            col = {}
            for k in col_names:
                col[k] = cp.tile([D, 1], f32, tag=k, name=k)
                nc.sync.dma_start(out=col[k][:], in_=col_d[k][:])
            if not lnh_trivial:
                ghb = cp.tile([D, D], f32, tag="ghb")
                bhb = cp.tile([D, D], f32, tag="bhb")
                nc.sync.dma_start(out=ghb[:], in_=ghb_d[:])
                nc.sync.dma_start(out=bhb[:], in_=bhb_d[:])
            sidx_t = cp.tile([128, epad // 16], i16, tag="sidx")
            didx_t = cp.tile([128, epad // 16], i16, tag="didx")
            nc.sync.dma_start(out=sidx_t[:], in_=sidx_d[:])
            nc.sync.dma_start(out=didx_t[:], in_=didx_d[:])
            hsT_t = cp.tile([128, NSLICE_PAD], f32, tag="hsT")
            nc.sync.dma_start(out=hsT_t[:], in_=hsT_d[:])
            agg_a = cp.tile([128, NGROUPS * 128], f32, tag="agg_a")
            agg_b = cp.tile([128, NGROUPS * 128], f32, tag="agg_b")
            nc.vector.memset(agg_a[:], 0.0)
            nc.vector.memset(agg_b[:], 0.0)
            c_col = cp.tile([D, 1], f32, tag="c_col")

            # ---- mlp_t -> c_col = emlp_b2 + tmlp_b2 + mlp_t_core ----
            with tc.tile_pool(name="ps0", bufs=1, space="PSUM") as ps0, \
                 tc.tile_pool(name="tp0", bufs=1) as tp0:
                t1 = ps0.tile([D, 1], f32, tag="t1")
                nc.tensor.matmul(t1[:], wf["tw1T"][:], col["tembc"][:],
                                 start=True, stop=True)
                y1t = tp0.tile([D, 1], f32, tag="y1t")
                nc.scalar.activation(y1t[:], t1[:], AF.Relu, bias=col["tb1c"][:])
                t2 = ps0.tile([D, 1], f32, tag="t2")
                nc.tensor.matmul(t2[:], wf["tw2T"][:], y1t[:],
                                 start=True, stop=True)
                nc.scalar.activation(c_col[:], t2[:], AF.Identity,
                                     bias=col["cb2c"][:])

            # ================= phase 1: edges =================
            with tc.tile_pool(name="chk", bufs=2) as chp, \
                 tc.tile_pool(name="tl", bufs=3) as tp, \
                 tc.tile_pool(name="psA", bufs=2, space="PSUM") as psA, \
                 tc.tile_pool(name="psB", bufs=1, space="PSUM") as psB:
                for ci in range(nch):
                    bank = ci // chunks_per_bank
                    c0 = ci * CHUNK
                    e_ch = chp.tile([128, CHUNK], bf16, tag="e_ch")
                    nc.sync.dma_start(out=e_ch[:], in_=eT_d[:, c0:c0 + CHUNK])
                    hsrcT = chp.tile([128, 1, CHUNK], bf16, tag="hsrcT")
                    hdstT = chp.tile([128, 1, CHUNK], bf16, tag="hdstT")
                    if SKIP_GATHERS:
                        nc.vector.memset(hsrcT[:], 0.0)
                        nc.vector.memset(hdstT[:], 0.0)
                    else:
                        nc.gpsimd.dma_gather(
                            hsrcT[:], hsb_d[:], sidx_t[:, c0 // 16:(c0 + CHUNK) // 16],
                            CHUNK, CHUNK, D, transpose=True,
                            single_packet=False)
                        nc.gpsimd.dma_gather(
                            hdstT[:], hbf_d[bank * BANK:bank * BANK + BANK],
                            didx_t[:, c0 // 16:(c0 + CHUNK) // 16],
                            CHUNK, CHUNK, D, transpose=True,
                            single_packet=False)
                    msg_ch = chp.tile([128, CHUNK // 128, D], f32, tag="msg_ch")
                    enew_ch = chp.tile([128, CHUNK], bf16, tag="enew_ch")

                    for t in range(CHUNK // TILE):
                        t0 = t * TILE
                        nsub = TILE // 128
                        ehat_ps = psA.tile([128, nsub, D], f32, tag="ehat_ps")
                        vh_ps = psA.tile([128, nsub, D], f32, tag="vh_ps")
                        for m in range(nsub):
                            s = t0 + m * 128
                            nc.tensor.matmul(ehat_ps[:, m, :], e_ch[:, s:s + 128],
                                             wb["PT"][:], start=True, stop=False)
                            nc.tensor.matmul(ehat_ps[:, m, :], hsrcT[:, 0, s:s + 128],
                                             wb["QT"][:], start=False, stop=False)
                            nc.tensor.matmul(ehat_ps[:, m, :], hdstT[:, 0, s:s + 128],
                                             wb["RT"][:], start=False, stop=True)
                            nc.tensor.matmul(vh_ps[:, m, :], hdstT[:, 0, s:s + 128],
                                             wb["VT"][:], start=True, stop=True)

                        # e_hat -> sbuf bf16 (ACT)
                        ehat_sb = tp.tile([128, nsub, D], bf16, tag="ehat_sb")
                        nc.scalar.activation(ehat_sb[:], ehat_ps[:], AF.Copy)
                        # LN stats (DVE)
                        st6 = tp.tile([128, nsub, 6], f32, tag="st6")
                        st2 = tp.tile([128, nsub, 2], f32, tag="st2")
                        for m in range(nsub):
                            nc.vector.bn_stats(st6[:, m, :], ehat_sb[:, m, :])
                            nc.vector.bn_aggr(st2[:, m, :], st6[:, m, :])
                        negm = tp.tile([128, nsub], f32, tag="negm")
                        nc.vector.tensor_scalar(negm[:], st2[:, :, 0], -1.0, None,
                                                ALU.mult)
                        std = tp.tile([128, nsub], f32, tag="std")
                        nc.scalar.activation(std[:], st2[:, :, 1], AF.Sqrt,
                                             bias=col["epsc"][:])
                        rinv = tp.tile([128, nsub], f32, tag="rinv")
                        nc.vector.reciprocal(rinv[:], std[:])
                        # LN apply (DVE): (x - m) * r
                        lne_sb = tp.tile([128, nsub, D], bf16, tag="lne_sb")
                        for m in range(nsub):
                            nc.vector.tensor_scalar(
                                lne_sb[:, m, :], ehat_sb[:, m, :],
                                negm[:, m:m + 1], rinv[:, m:m + 1],
                                ALU.add, ALU.mult)
                        # sigmoid (ACT), msg (DVE)
                        sig_sb = tp.tile([128, nsub, D], f32, tag="sig_sb")
                        nc.scalar.activation(sig_sb[:], ehat_ps[:], AF.Sigmoid)
                        nc.vector.tensor_tensor(
                            out=msg_ch[:, t0 // 128:t0 // 128 + nsub, :],
                            in0=sig_sb[:], in1=vh_ps[:], op=ALU.mult)
                        # transpose LNe -> [D, edges] (PE with identity rhs)
                        if SKIP_MLP:
                            nc.scalar.activation(enew_ch[:, t0:t0 + TILE],
                                                 ehat_ps[:], AF.Copy)
                            continue
                        lnet_ps = psB.tile([128, TILE], f32, tag="lnet_ps")
                        for m in range(nsub):
                            nc.tensor.matmul(lnet_ps[:, m * 128:(m + 1) * 128],
                                             lne_sb[:, m, :], wb["Ibf"][:],
                                             start=True, stop=True)
                        lnet_sb = tp.tile([128, TILE], bf16, tag="lnet_sb")
                        if t % 2 == 0:
                            nc.scalar.activation(lnet_sb[:], lnet_ps[:], AF.Copy)
                        else:
                            nc.vector.tensor_copy(lnet_sb[:], lnet_ps[:])
                        # mlp layer 1 (PE) + relu (ACT)
                        y1_ps = psB.tile([128, TILE], f32, tag="y1_ps")
                        nc.tensor.matmul(y1_ps[:], wb["w1T"][:], lnet_sb[:],
                                         start=True, stop=True)
                        y1_sb = tp.tile([128, TILE], bf16, tag="y1_sb")
                        nc.scalar.activation(y1_sb[:], y1_ps[:], AF.Relu,
                                             bias=col["b1c"][:])
                        # mlp layer 2 + e residual (PE), final bias (ACT)
                        enew_ps = psB.tile([128, TILE], f32, tag="enew_ps")
                        nc.tensor.matmul(enew_ps[:], wb["w2T"][:], y1_sb[:],
                                         start=True, stop=False)
                        nc.tensor.matmul(enew_ps[:], wb["Ibf"][:],
                                         e_ch[:, t0:t0 + TILE],
                                         start=False, stop=True)
                        nc.scalar.activation(enew_ch[:, t0:t0 + TILE], enew_ps[:],
                                             AF.Identity, bias=c_col[:])

                    # scatter msg into SBUF agg (parity CCE add); one call per
                    # WSCAT window — src indices are host-arranged unique
                    # within each window (HW races duplicate idxs in a call)
                    for w in range(0 if SKIP_SCATTER else CHUNK // WSCAT):
                        w0 = c0 + w * WSCAT
                        nc.gpsimd.dma_scatter_add(
                            agg_a[:],
                            msg_ch[:, w * WSCAT // 128:(w + 1) * WSCAT // 128, :],
                            sidx_t[:, w0 // 16:(w0 + WSCAT) // 16],
                            WSCAT, WSCAT, D,
                            sbuf_tokens_per_rank=128, parity_reg=0,
                            out_ap_other=agg_b[:], single_packet=False)
                    nc.sync.dma_start(out=enewT_d[:, c0:c0 + CHUNK], in_=enew_ch[:])

            # ================= phase 2: nodes =================
            with tc.tile_pool(name="p2", bufs=3) as p2, \
                 tc.tile_pool(name="ps2", bufs=2, space="PSUM") as ps2:
                for nt in range(0 if SKIP_PHASE2 else NSLICE_PAD // 128):
                    par, grp = nt & 1, nt >> 1
                    aggt = agg_a if par == 0 else agg_b
                    z_ps = ps2.tile([128, D], f32, tag="z_ps")
                    nc.tensor.matmul(z_ps[:], hsT_t[:, nt * 128:(nt + 1) * 128],
                                     wf["UT"][:], start=True, stop=True)
                    z2 = p2.tile([128, D], f32, tag="z2")
                    nc.vector.tensor_tensor(
                        out=z2[:], in0=z_ps[:],
                        in1=aggt[:, grp * 128:(grp + 1) * 128], op=ALU.add)
                    st6 = p2.tile([128, 6], f32, tag="p2st6")
                    nc.vector.bn_stats(st6[:], z2[:])
                    st2 = p2.tile([128, 2], f32, tag="p2st2")
                    nc.vector.bn_aggr(st2[:], st6[:])
                    negm = p2.tile([128, 1], f32, tag="p2negm")
                    nc.vector.tensor_scalar(negm[:], st2[:, 0:1], -1.0, None,
                                            ALU.mult)
                    std = p2.tile([128, 1], f32, tag="p2std")
                    nc.scalar.activation(std[:], st2[:, 1:2], AF.Sqrt,
                                         bias=col["epsc"][:])
                    rinv = p2.tile([128, 1], f32, tag="p2rinv")
                    nc.vector.reciprocal(rinv[:], std[:])
                    xn = p2.tile([128, D], f32, tag="xn")
                    nc.vector.tensor_scalar(xn[:], z2[:], negm[:], rinv[:],
                                            ALU.add, ALU.mult)
                    if not lnh_trivial:
                        nc.vector.tensor_tensor(out=xn[:], in0=xn[:], in1=ghb[:],
                                                op=ALU.mult)
                        nc.vector.tensor_tensor(out=xn[:], in0=xn[:], in1=bhb[:],
                                                op=ALU.add)
                    hs_t = p2.tile([128, D], f32, tag="hs_t")
                    nc.sync.dma_start(out=hs_t[:],
                                      in_=hs_d[nt * 128:(nt + 1) * 128, :])
                    out_sb = p2.tile([128, D], f32, tag="out_sb")
                    nc.vector.scalar_tensor_tensor(
                        out_sb[:], xn[:], 0.0, hs_t[:], ALU.max, ALU.add)
                    nc.sync.dma_start(out=hnew_d[nt * 128:(nt + 1) * 128, :],
                                      in_=out_sb[:])

    nc.compile()
    return nc


def _get_nc(eb_pad, lnh_trivial):
    key = (eb_pad, lnh_trivial)
    if key not in _cache:
        _cache[key] = _build(eb_pad, lnh_trivial)
    return _cache[key]


def prepare(h, e, src, dst, weights):
    """Host-side sharding/layout prep. Returns (in_maps, meta)."""
    eb_pad_min = 25600
    core = src // NSLICE
    bank = (dst >= BANK).astype(np.int64)
    group = core * 2 + bank
    counts = np.bincount(group, minlength=2 * NCORES)
    eb_pad = max(eb_pad_min, int(-(-counts.max() // CHUNK)) * CHUNK)
    epad = 2 * eb_pad

    # Assign each edge a slot such that, within every WSCAT-sized scatter
    # window of its (core, bank) segment, all src indices are unique
    # (the HW scatter-add races duplicate indices within one call).
    nw = eb_pad // WSCAT
    src_local_all = src - core * NSLICE
    pos = np.empty(E, np.int64)
    for c in range(NCORES):
        for b in range(2):
            g = c * 2 + b
            cell_base = c * epad + b * eb_pad
            idxs = np.nonzero(group == g)[0]
            sl = src_local_all[idxs]
            o = np.argsort(sl, kind="stable")
            sl_s, idxs_s = sl[o], idxs[o]
            starts = np.nonzero(np.r_[True, sl_s[1:] != sl_s[:-1]])[0]
            ends = np.r_[starts[1:], len(sl_s)]
            loads = np.zeros(nw, np.int64)
            for s0, s1 in zip(starts, ends):
                k = s1 - s0
                assert k <= nw, f"node with {k} edges in one (core,bank) cell"
                if k == nw:
                    wsel = np.arange(nw)
                else:
                    wsel = np.argpartition(loads, k - 1)[:k]
                for j in range(k):
                    w = wsel[j]
                    pos[idxs_s[s0 + j]] = cell_base + w * WSCAT + loads[w]
                    loads[w] += 1
            assert loads.max() <= WSCAT, f"scatter window overflow: {loads.max()}"

    slot_edge = np.full(NCORES * epad, -1, np.int64)
    slot_edge[pos] = np.arange(E)
    sel = slot_edge.reshape(NCORES, epad)
    valid = sel >= 0

    eT = np.zeros((NCORES, epad, D), np.float32)
    eT[valid] = e[sel[valid]]
    eT_bf = np.ascontiguousarray(eT.transpose(0, 2, 1)).astype(BF16)

    src_local = (src - core * NSLICE)
    sl = np.full((NCORES, epad), JUNK, np.int64)
    sl[valid] = src_local[sel[valid]]
    dst_local = dst - bank * BANK
    dl = np.zeros((NCORES, epad), np.int64)
    dl[valid] = dst_local[sel[valid]]
    sidx = _wrap16(sl)
    didx = _wrap16(dl)

    h_bf = np.ascontiguousarray(h.astype(BF16))
    hsb = np.zeros((NCORES, NSLICE_PAD, D), BF16)
    hsb[:, :NSLICE] = h.reshape(NCORES, NSLICE, D).astype(BF16)
    hsT = np.zeros((NCORES, 128, NSLICE_PAD), np.float32)
    hsT[:, :, :NSLICE] = h.reshape(NCORES, NSLICE, D).transpose(0, 2, 1)
    hs = np.zeros((NCORES, NSLICE_PAD, D), np.float32)
    hs[:, :NSLICE] = h.reshape(NCORES, NSLICE, D)

    w = weights
    g_e, b_e = w["ln_e_g"], w["ln_e_b"]
    w1p = w["emlp_w1"] * g_e[None, :]
    b1_eff = w["emlp_b1"] + w["emlp_w1"] @ b_e
    lnh_trivial = bool(np.all(w["ln_h_g"] == 1.0) and np.all(w["ln_h_b"] == 0.0))

    shared = {
        "hbf": h_bf,
        "PT": np.ascontiguousarray(w["P"].T).astype(BF16),
        "QT": np.ascontiguousarray(w["Q"].T).astype(BF16),
        "RT": np.ascontiguousarray(w["R"].T).astype(BF16),
        "VT": np.ascontiguousarray(w["V"].T).astype(BF16),
        "w1T": np.ascontiguousarray(w1p.T).astype(BF16),
        "w2T": np.ascontiguousarray(w["emlp_w2"].T).astype(BF16),
        "Ibf": np.eye(D, dtype=np.float32).astype(BF16),
        "UT": np.ascontiguousarray(w["U"].T).astype(np.float32),
        "tw1T": np.ascontiguousarray(w["tmlp_w1"].T).astype(np.float32),
        "tw2T": np.ascontiguousarray(w["tmlp_w2"].T).astype(np.float32),
        "b1c": b1_eff.reshape(D, 1).astype(np.float32),
        "tembc": np.ascontiguousarray(w["t_emb"].reshape(1, D).T).astype(np.float32),
        "tb1c": w["tmlp_b1"].reshape(D, 1).astype(np.float32),
        "cb2c": (w["tmlp_b2"] + w["emlp_b2"]).reshape(D, 1).astype(np.float32),
        "epsc": np.full((D, 1), 1e-5, np.float32),
    }
    if not lnh_trivial:
        shared["ghb"] = np.broadcast_to(
            w["ln_h_g"].astype(np.float32), (D, D)).copy()
        shared["bhb"] = np.broadcast_to(
            w["ln_h_b"].astype(np.float32), (D, D)).copy()

    in_maps = []
    for c in range(NCORES):
        m = dict(shared)
        m["eT"] = eT_bf[c]
        m["hsb"] = hsb[c]
        m["sidx"] = sidx[c]
        m["didx"] = didx[c]
        m["hsT"] = hsT[c]
        m["hs"] = hs[c]
        in_maps.append(m)
    return in_maps, dict(eb_pad=eb_pad, epad=epad, pos=pos,
                         lnh_trivial=lnh_trivial)


def kernel(h, e, edge_index, t_emb, P, Q, R, U, V, ln_e_g, ln_e_b,
           emlp_w1, emlp_b1, emlp_w2, emlp_b2,
           tmlp_w1, tmlp_b1, tmlp_w2, tmlp_b2, ln_h_g, ln_h_b):
    h = np.asarray(h, np.float32)
    e = np.asarray(e, np.float32)
    ei = np.asarray(edge_index)
    src = ei[0].astype(np.int64)
    dst = ei[1].astype(np.int64)
    weights = dict(
        t_emb=np.asarray(t_emb, np.float32), P=np.asarray(P, np.float32),
        Q=np.asarray(Q, np.float32), R=np.asarray(R, np.float32),
        U=np.asarray(U, np.float32), V=np.asarray(V, np.float32),
        ln_e_g=np.asarray(ln_e_g, np.float32), ln_e_b=np.asarray(ln_e_b, np.float32),
        emlp_w1=np.asarray(emlp_w1, np.float32), emlp_b1=np.asarray(emlp_b1, np.float32),
        emlp_w2=np.asarray(emlp_w2, np.float32), emlp_b2=np.asarray(emlp_b2, np.float32),
        tmlp_w1=np.asarray(tmlp_w1, np.float32), tmlp_b1=np.asarray(tmlp_b1, np.float32),
        tmlp_w2=np.asarray(tmlp_w2, np.float32), tmlp_b2=np.asarray(tmlp_b2, np.float32),
        ln_h_g=np.asarray(ln_h_g, np.float32), ln_h_b=np.asarray(ln_h_b, np.float32),
    )
    in_maps, meta = prepare(h, e, src, dst, weights)
    nc = _get_nc(meta["eb_pad"], meta["lnh_trivial"])

    from concourse.bass_utils import run_bass_kernel_spmd
    res = run_bass_kernel_spmd(nc, in_maps, list(range(NCORES))).results

    epad = meta["epad"]
    flat = np.concatenate(
        [np.asarray(res[c]["enewT"]).astype(np.float32).T for c in range(NCORES)],
        axis=0)                                            # [NCORES*epad, D]
    e_new = flat[meta["pos"]]
    h_new = np.concatenate(
        [np.asarray(res[c]["hnew"])[:NSLICE] for c in range(NCORES)], axis=0)
    return h_new.astype(np.float32), e_new.astype(np.float32)
